# revision 1
# baseline (speedup 1.0000x reference)
"""BraggNN Trainium2 kernel (8-core data-parallel, Bass/Tile).

Strategy:
  - Feature-major layout: features on SBUF partitions, batch on the free dim.
  - Every conv becomes a block-sparse Toeplitz matmul with host-precomputed
    (deduplicated) fp32->TF32 weight blocks; spatial rows are padded to
    power-friendly widths so conv2/conv3 blocks repeat and dedup tightly.
  - conv1 is composed into the NLB 1x1 convs on the host (theta/phi/g read x
    directly); the NLB residual is realized by accumulating the conv1 matmul
    and the W_o matmul into the same PSUM bank, evacuated by a single
    bias+LeakyReLU activation op.
  - softmax over W: exp on ScalarE, row-sums and 1/sum expansion via ones
    matmuls on the TensorE, reciprocal on VectorE.
  - All matmuls run as float32r (TF32): 1 cycle/row, fp32 accumulate.
"""

import os
import sys

for _p in ("/opt/trn_rl_repo", "/root/.axon_site/_ro/trn_rl_repo"):
    if os.path.isdir(_p) and _p not in sys.path:
        sys.path.insert(0, _p)

import numpy as np

# ----------------------------------------------------------------------------
# Geometry (hardcoded for BraggNN: x [B,1,11,11], B=16384)
# ----------------------------------------------------------------------------
B_TOTAL = 16384
N_CORES = 8
B_CORE = B_TOTAL // N_CORES          # 2048
BT = int(os.environ.get("KBT", "512"))   # batch tile (free dim per op)
NBT = B_CORE // BT                    # 8

# grid1 / h-space: conv1 output 9x9, padded cols 9->10 => 90 positions, 64 ch
G1_R, G1_C, G1_CP = 9, 9, 10
NPOS1 = G1_R * G1_CP                  # 90
HF = NPOS1 * 64                       # 5760 features, 45 tiles of 128
HT = HF // 128                        # 45

# s-space: NLB inter space, 32 ch over grid1
SF = NPOS1 * 32                       # 2880
ST = (SF + 127) // 128                # 23 tiles (last uses 64 partitions)

# sums space: one value per (row i, c') => 9*32 = 288, chunks of 3 rows = 96
SUMF = G1_R * 32                      # 288
SUM_CHUNK = 32                        # 1 row per chunk
NSUM = SUMF // SUM_CHUNK              # 9

# grid2 / conv2 out: 7x7 valid, padded cols 7->8 => 56 positions, 32 ch
G2_R, G2_C, G2_CP = 7, 7, 8
NPOS2 = G2_R * G2_CP                  # 56
C2F = NPOS2 * 32                      # 1792
C2T = C2F // 128                      # 14

# grid3 / conv3 out: 5x5 valid, padded cols 5->6 => 30 positions, 8 ch
G3_R, G3_C, G3_CP = 5, 5, 6
NPOS3 = G3_R * G3_CP                  # 30
C3F = NPOS3 * 8                       # 240
C3T = 2                               # tiles: [128, 112]

XF = 121                              # input features 11*11


def tf32_round(a):
    u = np.ascontiguousarray(a, dtype=np.float32).view(np.uint32)
    u = (u + np.uint32(0x0FFF) + ((u >> np.uint32(13)) & np.uint32(1))) & np.uint32(0xFFFFE000)
    return u.view(np.float32)


def _p1(i, j):
    return i * G1_CP + j


def _p2(i, j):
    return i * G2_CP + j


def _p3(i, j):
    return i * G3_CP + j


# ----------------------------------------------------------------------------
# Host-side construction of all full (dense) layer matrices + bias vectors
# ----------------------------------------------------------------------------
def build_full_mats(inp):
    w1, b1 = inp["w1"], inp["b1"]          # [64,1,3,3], [64]
    wt, bt = inp["wt"][:, :, 0, 0], inp["bt"]
    wp, bp = inp["wp"][:, :, 0, 0], inp["bp"]
    wg, bg = inp["wg"][:, :, 0, 0], inp["bg"]
    wo, bo = inp["wo"][:, :, 0, 0], inp["bo"]
    w2, b2 = inp["w2"], inp["b2"]          # [32,64,3,3]
    w3, b3 = inp["w3"], inp["b3"]          # [8,32,3,3]

    M = {}
    # conv1: x [121] -> h [5760]
    W1 = np.zeros((XF, HF), np.float32)
    for i in range(G1_R):
        for j in range(G1_C):
            p = _p1(i, j) * 64
            for ki in range(3):
                for kj in range(3):
                    W1[(i + ki) * 11 + (j + kj), p:p + 64] = w1[:, 0, ki, kj]
    M["W1"] = W1
    # bias for the fused conv1+wo evac: (b1 + bo) at real positions
    bh = np.zeros(HF, np.float32)
    for i in range(G1_R):
        for j in range(G1_C):
            bh[_p1(i, j) * 64:_p1(i, j) * 64 + 64] = b1 + bo
    M["bh"] = bh

    # composed theta/phi/g: x [121] -> s [2880]; eff 3x3 conv with 32 out ch
    for name, wmat, bvec in (("T", wt, bt), ("P", wp, bp), ("G", wg, bg)):
        wcomp = np.einsum("oc,ckl->okl", wmat, w1[:, 0])   # [32,3,3]
        beff = bvec + wmat @ b1                             # [32]
        Wf = np.zeros((XF, SF), np.float32)
        bf = np.zeros(SF, np.float32)
        for i in range(G1_R):
            for j in range(G1_C):
                p = _p1(i, j) * 32
                bf[p:p + 32] = beff
                for ki in range(3):
                    for kj in range(3):
                        Wf[(i + ki) * 11 + (j + kj), p:p + 32] = wcomp[:, ki, kj]
        M["W" + name] = Wf
        M["b" + name] = bf

    # ones for row sums: s [2880] -> sums [288]
    ONES = np.zeros((SF, SUMF), np.float32)
    for i in range(G1_R):
        for j in range(G1_C):
            for c in range(32):
                ONES[_p1(i, j) * 32 + c, i * 32 + c] = 1.0
    M["ONES"] = ONES
    M["EXP"] = ONES.T.copy()               # sums [288] -> s [2880]

    # wo: ag [2880] -> h [5760]
    WO = np.zeros((SF, HF), np.float32)
    for i in range(G1_R):
        for j in range(G1_C):
            p = _p1(i, j)
            WO[p * 32:p * 32 + 32, p * 64:p * 64 + 64] = wo.T
    M["WO"] = WO

    # conv2: h [5760] -> c2 [1792]
    W2 = np.zeros((HF, C2F), np.float32)
    b2f = np.zeros(C2F, np.float32)
    for i in range(G2_R):
        for j in range(G2_C):
            p = _p2(i, j) * 32
            b2f[p:p + 32] = b2
            for ki in range(3):
                for kj in range(3):
                    q = _p1(i + ki, j + kj) * 64
                    W2[q:q + 64, p:p + 32] = w2[:, :, ki, kj].T
    M["W2"] = W2
    M["b2"] = b2f

    # conv3: c2 [1792] -> c3 [240]
    W3 = np.zeros((C2F, C3F), np.float32)
    b3f = np.zeros(C3F, np.float32)
    for i in range(G3_R):
        for j in range(G3_C):
            p = _p3(i, j) * 8
            b3f[p:p + 8] = b3
            for ki in range(3):
                for kj in range(3):
                    q = _p2(i + ki, j + kj) * 32
                    W3[q:q + 32, p:p + 8] = w3[:, :, ki, kj].T
    M["W3"] = W3
    M["b3"] = b3f

    # dense head; dw1 permuted from torch (c,i,j) flatten to our padded layout
    D1 = np.zeros((C3F, 64), np.float32)
    for c in range(8):
        for i in range(G3_R):
            for j in range(G3_C):
                D1[_p3(i, j) * 8 + c, :] = inp["dw1"][:, c * 25 + i * 5 + j]
    M["D1"] = D1
    M["D2"] = inp["dw2"].T.copy()
    M["D3"] = inp["dw3"].T.copy()
    D4 = np.zeros((16, 16), np.float32)
    D4[:, :8] = inp["dw4"].T
    M["D4"] = D4
    D5 = np.zeros((16, 8), np.float32)
    D5[:8, :2] = inp["dw5"].T
    M["D5"] = D5
    for k in range(1, 4):
        M["bd%d" % k] = inp["db%d" % k].astype(np.float32)
    bd4 = np.zeros(16, np.float32)
    bd4[:8] = inp["db4"]
    M["bd4"] = bd4
    bd5 = np.zeros(8, np.float32)
    bd5[:2] = inp["db5"]
    M["bd5"] = bd5
    return M


# ----------------------------------------------------------------------------
# Numpy forward using the full matrices (layout validator)
# ----------------------------------------------------------------------------
def np_forward(M, xcols):
    """xcols: [121, N] feature-major input. Returns [2, N]."""
    lrelu = lambda v: np.where(v >= 0, v, 0.01 * v)
    th = M["WT"].T @ xcols + M["bT"][:, None]
    ph = M["WP"].T @ xcols + M["bP"][:, None]
    gg = M["WG"].T @ xcols + M["bG"][:, None]
    es = np.exp(th * ph)
    sums = M["ONES"].T @ es
    rcp = 1.0 / sums
    ag = es * gg * (M["EXP"].T @ rcp)
    h = M["W1"].T @ xcols + M["WO"].T @ ag
    h = lrelu(h + M["bh"][:, None])
    c2 = lrelu(M["W2"].T @ h + M["b2"][:, None])
    c3 = lrelu(M["W3"].T @ c2 + M["b3"][:, None])
    z = lrelu(M["D1"].T @ c3 + M["bd1"][:, None])
    z = lrelu(M["D2"].T @ z + M["bd2"][:, None])
    z = lrelu(M["D3"].T @ z + M["bd3"][:, None])
    z = lrelu(M["D4"].T @ z + M["bd4"][:, None])
    return (M["D5"].T @ z + M["bd5"][:, None])[:2]


# ----------------------------------------------------------------------------
# Block decomposition with dedup
# ----------------------------------------------------------------------------
class BlockBank:
    """Collects [K<=128, M<=128] lhsT blocks into one [128, total] blob."""

    def __init__(self, bank_id):
        self.bank_id = bank_id
        self.cols = []          # list of np [128, m] blocks
        self.total = 0
        self.index = {}         # bytes -> (bank, wid_offset, K, M)

    def add(self, blk):
        """blk: [K, M] np.float32. Returns (bank, col_offset, K, M)."""
        K, Mm = blk.shape
        key = (K, Mm, blk.tobytes())
        hit = self.index.get(key)
        if hit is not None:
            return hit
        pad = np.zeros((128, Mm), np.float32)
        pad[:K] = blk
        ent = (self.bank_id, self.total, K, Mm)
        self.cols.append(pad)
        self.total += Mm
        self.index[key] = ent
        return ent

    def blob(self):
        return np.concatenate(self.cols, axis=1) if self.cols else np.zeros((128, 0), np.float32)


def decompose(bank, full, k_tiles, m_tiles):
    """full: [Kdim, Mdim]. k_tiles/m_tiles: lists of (lo, hi) ranges.
    Returns per m-tile a list of (k_idx, (off, K, M)) skipping zero blocks."""
    out = []
    for (mlo, mhi) in m_tiles:
        ents = []
        for ki, (klo, khi) in enumerate(k_tiles):
            blk = full[klo:khi, mlo:mhi]
            if not np.any(blk):
                continue
            ents.append((ki, bank.add(np.ascontiguousarray(blk))))
        out.append(ents)
    return out


def tiles_of(nfeat, tile=128):
    return [(lo, min(lo + tile, nfeat)) for lo in range(0, nfeat, tile)]


class BiasBank:
    def __init__(self):
        self.cols = []
        self.index = {}

    def add(self, vec):
        """vec: [P] np.float32 -> (col, P)"""
        P = vec.shape[0]
        key = (P, vec.tobytes())
        hit = self.index.get(key)
        if hit is not None:
            return hit
        pad = np.zeros(128, np.float32)
        pad[:P] = vec
        ent = (len(self.cols), P)
        self.cols.append(pad)
        self.index[key] = ent
        return ent

    def blob(self):
        return (np.stack(self.cols, axis=1) if self.cols
                else np.zeros((128, 0), np.float32))


def build_plan(inp):
    """Returns (plan, wblob, bblob). plan holds all block tables."""
    M = build_full_mats(inp)
    bank = BlockBank(0)      # float32r sections: tpg, conv1, expand
    bankb = BlockBank(1)     # bf16 sections: ones, wo, conv2, conv3, dense
    bias = BiasBank()
    P = {}

    xt = [(0, XF)]
    st = tiles_of(SF)
    ht = tiles_of(HF)
    sumt = [(k * SUM_CHUNK, (k + 1) * SUM_CHUNK) for k in range(NSUM)]
    c2t = tiles_of(C2F)
    c3t = tiles_of(C3F)

    # tpg: one block per (tensor, s-tile), K = x
    for nm in ("T", "P", "G"):
        P["tpg" + nm] = decompose(bank, M["W" + nm], xt, st)
        P["bias" + nm] = [bias.add(M["b" + nm][lo:hi]) for (lo, hi) in st]
    # ones: K = s-tiles, M = sums chunks  (stored per K for accumulation order)
    P["ones"] = decompose(bankb, M["ONES"], st, sumt)
    # expand: K = sums chunks, M = s-tiles
    P["expand"] = decompose(bank, M["EXP"], sumt, st)
    # conv1: K = x, M = h-tiles
    P["conv1"] = decompose(bank, M["W1"], xt, ht)
    # wo: K = s-tiles, M = h-tiles
    P["wo"] = decompose(bankb, M["WO"], st, ht)
    P["biasH"] = [bias.add(M["bh"][lo:hi]) for (lo, hi) in ht]
    # conv2: K = h-tiles, M = c2 tiles
    P["conv2"] = decompose(bankb, M["W2"], ht, c2t)
    P["bias2"] = [bias.add(M["b2"][lo:hi]) for (lo, hi) in c2t]
    # conv3: K = c2 tiles, M = c3 tiles
    P["conv3"] = decompose(bankb, M["W3"], c2t, c3t)
    P["bias3"] = [bias.add(M["b3"][lo:hi]) for (lo, hi) in c3t]
    # dense
    P["d1"] = decompose(bankb, M["D1"], c3t, [(0, 64)])
    P["d2"] = decompose(bankb, M["D2"], [(0, 64)], [(0, 32)])
    P["d3"] = decompose(bankb, M["D3"], [(0, 32)], [(0, 16)])
    P["d4"] = decompose(bankb, M["D4"], [(0, 16)], [(0, 16)])
    P["d5"] = decompose(bankb, M["D5"], [(0, 16)], [(0, 8)])
    for k in range(1, 6):
        P["biasd%d" % k] = bias.add(M["bd%d" % k])

    # sums chunk schedule: for each s-tile u, which chunks it feeds; and
    # per chunk the ordered list of contributing u (for start/stop flags)
    contrib = [[] for _ in range(NSUM)]
    for mi, ents in enumerate(P["ones"]):
        pass
    # P["ones"][chunk] is list over chunks; reorganize per (u -> [(chunk, ent)])
    per_u = [[] for _ in range(ST)]
    for ch, ents in enumerate(P["ones"]):
        for (u, ent) in ents:
            per_u[u].append((ch, ent))
            contrib[ch].append(u)
    P["ones_per_u"] = per_u
    P["sums_first_u"] = [min(c) for c in contrib]
    P["sums_last_u"] = [max(c) for c in contrib]
    # expand: per s-tile u, list of (chunk, ent)
    P["expand_per_u"] = [list(ents) for ents in P["expand"]]
    P["kmax_u"] = [max(ch for ch, _ in ents) for ents in P["expand_per_u"]]

    return P, (bank.blob(), bankb.blob()), bias.blob(), M


# ----------------------------------------------------------------------------
# Bass kernel emission
# ----------------------------------------------------------------------------
DBG_STAGE = 9          # 1=tpg/sums, 2=+attn, 3=+conv2, 9=full
DBG_STU = None         # limit number of s-tiles
DBG_LOOP = 0           # device-side repeat count for benchmarking
import json as _json
TUNE = {"php": 3, "s": 3, "a1": 2, "ag": 3, "es": 5, "gp": 5, "h2": 16,
        "c2": 11, "mm": 6, "x": 2, "a1_eng": "dve", "gp_eng": "dve",
        "rcp": 3}
if os.environ.get("KTUNE"):
    TUNE.update(_json.loads(os.environ["KTUNE"]))
DBG_NO_SUMS = False    # skip sums/rcp emission
DBG_NBT = None         # override batch-tile count for bisection


def emit_bass(plan, wcols, bcols):
    wcols_a, wcols_b = wcols
    import concourse.bacc as bacc
    import concourse.mybir as mybir
    from concourse.tile import TileContext

    F32R = mybir.dt.float32r
    F32 = mybir.dt.float32
    AF = mybir.ActivationFunctionType
    OP = mybir.AluOpType
    P = plan

    import os as _os
    nd = int(_os.environ.get("DBG_ND", str(N_CORES)))
    import os as _os2
    tbl = _os2.environ.get("DBG_TBL", "1") == "1"
    nc = bacc.Bacc("TRN2", target_bir_lowering=tbl, debug=False,
                   num_devices=nd)
    BF16 = mybir.dt.bfloat16
    x_d = nc.dram_tensor("x", [XF, B_CORE], F32R, kind="ExternalInput")
    w_d = nc.dram_tensor("wb", [128, wcols_a], F32R, kind="ExternalInput")
    w2_d = nc.dram_tensor("wb2", [128, wcols_b], BF16, kind="ExternalInput")
    b_d = nc.dram_tensor("bb", [128, bcols], F32, kind="ExternalInput")
    y_d = nc.dram_tensor("y", [2, B_CORE], F32, kind="ExternalOutput")

    st = tiles_of(SF)
    ht = tiles_of(HF)
    c2t = tiles_of(C2F)
    c3t = tiles_of(C3F)

    with TileContext(nc) as tc:
        with nc.allow_low_precision(reason="TF32 activations by design"), \
             tc.tile_pool(name="sb", bufs=1) as sb, \
             tc.tile_pool(name="ps", bufs=1, space="PSUM") as psp:

            # ---- weights/biases resident in SBUF ----
            wsb = sb.tile([128, wcols_a], F32R, tag="wsb", bufs=1)
            wsb2 = sb.tile([128, wcols_b], BF16, tag="wsb2", bufs=1)
            bsb = sb.tile([128, bcols], F32, tag="bsb", bufs=1)
            CH = 2048
            for lo in range(0, wcols_a, CH):
                hi = min(lo + CH, wcols_a)
                nc.sync.dma_start(out=wsb[:, lo:hi], in_=w_d[:, lo:hi])
            for lo in range(0, wcols_b, CH):
                hi = min(lo + CH, wcols_b)
                nc.sync.dma_start(out=wsb2[:, lo:hi], in_=w2_d[:, lo:hi])
            nc.sync.dma_start(out=bsb[:], in_=b_d[:])

            def wap(ent):
                bk, off, K, Mm = ent
                base = wsb if bk == 0 else wsb2
                return base[0:K, off:off + Mm]

            def bap(ent):
                col, Pp = ent
                return bsb[0:Pp, col:col + 1]

            def mm_chain(ps_ap, ents, rhs_of):
                n = len(ents)
                for idx, (ki, ent) in enumerate(ents):
                    nc.tensor.matmul(ps_ap, wap(ent), rhs_of(ki),
                                     start=(idx == 0), stop=(idx == n - 1))

            nbt = DBG_NBT or NBT
            import contextlib as _ctx
            loop_cm = (tc.For_i(0, DBG_LOOP, 1,
                                hint_engines=(mybir.EngineType.PE,
                                              mybir.EngineType.Activation,
                                              mybir.EngineType.DVE))
                       if DBG_LOOP > 1 else _ctx.nullcontext())
            with loop_cm:
              for bt in range(nbt):
                  bsl = slice(bt * BT, (bt + 1) * BT)
                  x_sb = sb.tile([XF, BT], F32R, tag="x", bufs=TUNE["x"], name="x_sb")
                  nc.sync.dma_start(out=x_sb[:], in_=x_d[:, bsl])

                  es = [None] * ST
                  gp = [None] * ST
                  h2 = [None] * HT
                  c2 = [None] * C2T
                  sums_ps = [None] * NSUM
                  rcp = [None] * NSUM
                  attn_done = [False] * ST
                  c2_done = [False] * C2T

                  def emit_c2_ready():
                      if DBG_STAGE < 3:
                          return
                      # emit any conv2 output tile whose h2 inputs all exist
                      for ot in range(C2T):
                          if c2_done[ot]:
                              continue
                          if any(h2[ki] is None for ki, _ in P["conv2"][ot]):
                              continue
                          cps = psp.tile([128, BT], F32, tag="mm", bufs=TUNE["mm"],
                                         name="cps")
                          mm_chain(cps[:], P["conv2"][ot],
                                   lambda ki: h2[ki][:])
                          c2m = sb.tile([128, BT], BF16, tag="c2", bufs=TUNE["c2"],
                                        name="c2t")
                          nc.scalar.activation(c2m[:], cps[:], AF.Lrelu,
                                               bias=bap(P["bias2"][ot]),
                                               alpha=0.01)
                          c2[ot] = c2m
                          c2_done[ot] = True

                  def emit_attn(u):
                      if DBG_STAGE < 2:
                          return
                      lo, hi = st[u]
                      Mu = hi - lo
                      ep = psp.tile([128, BT], F32, tag="mm", bufs=TUNE["mm"], name="ep")
                      ents = P["expand_per_u"][u]
                      for idx, (ch, ent) in enumerate(ents):
                          nc.tensor.matmul(ep[0:Mu, :], wap(ent), rcp[ch][:],
                                           start=(idx == 0),
                                           stop=(idx == len(ents) - 1))
                      a1 = sb.tile([Mu, BT], BF16, tag="a1", bufs=TUNE["a1"], name="a1")
                      a1_eng = nc.gpsimd if TUNE["a1_eng"] == "gps" else nc.vector
                      a1_eng.tensor_tensor(out=a1[:], in0=es[u][0:Mu, :],
                                           in1=gp[u][0:Mu, :], op=OP.mult)
                      ag = sb.tile([Mu, BT], BF16, tag="ag", bufs=TUNE["ag"], name="ag")
                      nc.vector.tensor_tensor(out=ag[:], in0=a1[:],
                                              in1=ep[0:Mu, :], op=OP.mult)
                      for m in (2 * u, 2 * u + 1):
                          if m >= HT:
                              continue
                          hps = psp.tile([128, BT], F32, tag="mm", bufs=TUNE["mm"], name="hps")
                          (kx, ent1) = P["conv1"][m][0]
                          nc.tensor.matmul(hps[:], wap(ent1), x_sb[:],
                                           start=True, stop=False)
                          (ku, ent2) = P["wo"][m][0]
                          assert ku == u
                          woap = wap(ent2)
                          nc.tensor.matmul(hps[:], woap,
                                           ag[0:woap.shape[0], :],
                                           start=False, stop=True)
                          h2m = sb.tile([128, BT], BF16, tag="h2", bufs=TUNE["h2"],
                                        name="h2t")
                          nc.scalar.activation(h2m[:], hps[:], AF.Lrelu,
                                               bias=bap(P["biasH"][m]),
                                               alpha=0.01)
                          h2[m] = h2m
                      emit_c2_ready()

                  # ---------- phase T: tpg + exp + sums ----------
                  for u in range(DBG_STU or ST):
                      lo, hi = st[u]
                      Mu = hi - lo
                      pps = psp.tile([128, BT], F32, tag="mm", bufs=TUNE["mm"], name="pps")
                      (_, entP) = P["tpgP"][u][0]
                      nc.tensor.matmul(pps[0:Mu, :], wap(entP), x_sb[:],
                                       start=True, stop=True)
                      tps = psp.tile([128, BT], F32, tag="mm", bufs=TUNE["mm"], name="tps")
                      (_, entT) = P["tpgT"][u][0]
                      nc.tensor.matmul(tps[0:Mu, :], wap(entT), x_sb[:],
                                       start=True, stop=True)
                      gps = psp.tile([128, BT], F32, tag="mm", bufs=TUNE["mm"], name="gps")
                      (_, entG) = P["tpgG"][u][0]
                      nc.tensor.matmul(gps[0:Mu, :], wap(entG), x_sb[:],
                                       start=True, stop=True)

                      php = sb.tile([Mu, BT], BF16, tag="php", bufs=TUNE["php"], name="php")
                      nc.scalar.activation(php[:], pps[0:Mu, :], AF.Identity,
                                           bias=bap(P["biasP"][u]))
                      gpu = sb.tile([Mu, BT], BF16, tag="gp", bufs=TUNE["gp"], name="gpt")
                      if TUNE["gp_eng"] == "act":
                          nc.scalar.activation(gpu[:], gps[0:Mu, :], AF.Identity,
                                               bias=bap(P["biasG"][u]))
                      else:
                          nc.vector.tensor_scalar_add(gpu[:], gps[0:Mu, :],
                                                      bap(P["biasG"][u]))
                      gp[u] = gpu
                      s_sb = sb.tile([Mu, BT], BF16, tag="s", bufs=TUNE["s"], name="s_sb")
                      nc.vector.scalar_tensor_tensor(
                          out=s_sb[:], in0=tps[0:Mu, :],
                          scalar=bap(P["biasT"][u]), in1=php[:],
                          op0=OP.add, op1=OP.mult)
                      esu = sb.tile([Mu, BT], BF16, tag="es", bufs=TUNE["es"], name="est")
                      nc.scalar.activation(esu[:], s_sb[:], AF.Exp)
                      es[u] = esu

                      for (ch, ent) in ([] if DBG_NO_SUMS else P["ones_per_u"][u]):
                          if sums_ps[ch] is None:
                              sums_ps[ch] = psp.tile([SUM_CHUNK, BT], F32,
                                                     tag="sums", bufs=TUNE.get("sums", 2),
                                                     name="sums_ps")
                          nc.tensor.matmul(
                              sums_ps[ch][:], wap(ent), esu[:],
                              start=(u == P["sums_first_u"][ch]),
                              stop=(u == P["sums_last_u"][ch]))

                      for ch in range(0 if DBG_NO_SUMS else NSUM):
                          if rcp[ch] is None and P["sums_last_u"][ch] == u:
                              r = sb.tile([SUM_CHUNK, BT], F32R, tag="rcp",
                                          bufs=TUNE["rcp"], name="rcp_sb")
                              nc.vector.reciprocal(r[:], sums_ps[ch][:])
                              rcp[ch] = r
                              for u2 in range(u + 1):
                                  if (not attn_done[u2]
                                          and P["kmax_u"][u2] <= ch
                                          and rcp[P["kmax_u"][u2]] is not None):
                                      emit_attn(u2)
                                      attn_done[u2] = True

                  if DBG_STAGE < 2:
                      y_sb0 = sb.tile([2, BT], F32, tag="y", bufs=2, name="ydbg")
                      src_dbg = es[0] if DBG_NO_SUMS else rcp[2]
                      nc.vector.tensor_copy(y_sb0[:], src_dbg[0:2, :])
                      nc.sync.dma_start(out=y_d[:, bsl], in_=y_sb0[:])
                      continue
                  for u in range(ST):
                      if not attn_done[u]:
                          emit_attn(u)
                          attn_done[u] = True

                  if DBG_STAGE < 3:
                      y_sb0 = sb.tile([2, BT], F32, tag="y", bufs=2, name="ydbg")
                      nc.vector.tensor_copy(y_sb0[:], h2[44][0:2, :])
                      nc.sync.dma_start(out=y_d[:, bsl], in_=y_sb0[:])
                      continue
                  emit_c2_ready()
                  assert all(c2_done)
                  if DBG_STAGE < 4:
                      y_sb0 = sb.tile([2, BT], F32, tag="y", bufs=2, name="ydbg")
                      nc.vector.tensor_copy(y_sb0[:], c2[13][0:2, :])
                      nc.sync.dma_start(out=y_d[:, bsl], in_=y_sb0[:])
                      continue

                  # ---------- conv3 ----------
                  c3 = [None] * C3T
                  for ot in range(C3T):
                      lo, hi = c3t[ot]
                      Mo = hi - lo
                      cps = psp.tile([128, BT], F32, tag="mm", bufs=TUNE["mm"], name="c3ps")
                      mm_chain(cps[0:Mo, :], P["conv3"][ot],
                               lambda ki: c2[ki][:])
                      c3m = sb.tile([Mo, BT], BF16, tag="c3", bufs=2, name="c3t")
                      nc.scalar.activation(c3m[:], cps[0:Mo, :], AF.Lrelu,
                                           bias=bap(P["bias3"][ot]), alpha=0.01)
                      c3[ot] = c3m

                  if DBG_STAGE < 5:
                      y_sb0 = sb.tile([2, BT], F32, tag="y", bufs=2, name="ydbg")
                      nc.vector.tensor_copy(y_sb0[:], c3[1][0:2, :])
                      nc.sync.dma_start(out=y_d[:, bsl], in_=y_sb0[:])
                      continue
                  # ---------- dense head ----------
                  def dense(nm, rhs_tiles, Mo, func, bias_ent, tag, dt=None):
                      dt = dt or BF16
                      dps = psp.tile([Mo, BT], F32, tag="mm", bufs=TUNE["mm"], name="dps")
                      mm_chain(dps[:], P[nm][0], lambda ki: rhs_tiles[ki][:])
                      z = sb.tile([Mo, BT], dt, tag="z", bufs=3, name="z" + nm)
                      nc.scalar.activation(z[:], dps[:], func,
                                           bias=bap(bias_ent),
                                           alpha=0.01 if func == AF.Lrelu else 0.0)
                      return z

                  z1 = dense("d1", c3, 64, AF.Lrelu, P["biasd1"], "z1")
                  if DBG_STAGE < 6:
                      y_sb0 = sb.tile([2, BT], F32, tag="y", bufs=2, name="ydbg")
                      nc.vector.tensor_copy(y_sb0[:], z1[0:2, :])
                      nc.sync.dma_start(out=y_d[:, bsl], in_=y_sb0[:])
                      continue
                  z2 = dense("d2", [z1], 32, AF.Lrelu, P["biasd2"], "z2")
                  if DBG_STAGE < 7:
                      y_sb0 = sb.tile([2, BT], F32, tag="y", bufs=2, name="ydbg")
                      nc.vector.tensor_copy(y_sb0[:], z2[0:2, :])
                      nc.sync.dma_start(out=y_d[:, bsl], in_=y_sb0[:])
                      continue
                  z3 = dense("d3", [z2], 16, AF.Lrelu, P["biasd3"], "z3")
                  z4 = dense("d4", [z3], 16, AF.Lrelu, P["biasd4"], "z4")
                  if DBG_STAGE < 8:
                      y_sb0 = sb.tile([2, BT], F32, tag="y", bufs=2, name="ydbg")
                      nc.vector.tensor_copy(y_sb0[:], z4[0:2, :])
                      nc.sync.dma_start(out=y_d[:, bsl], in_=y_sb0[:])
                      continue
                  y_sb = dense("d5", [z4], 8, AF.Identity, P["biasd5"], "y",
                               dt=F32)
                  if DBG_STAGE < 9:
                      y_sb0 = sb.tile([2, BT], F32, tag="y2", bufs=2, name="ydbg2")
                      nc.vector.tensor_copy(y_sb0[:, 0:128], y_sb[0:2, 0:128])
                      nc.vector.tensor_copy(y_sb0[:, 128:256], z4[0:2, 128:256])
                      nc.sync.dma_start(out=y_d[:, bsl], in_=y_sb0[:])
                      continue
                  nc.sync.dma_start(out=y_d[:, bsl], in_=y_sb[0:2, :])
    if not nc.is_finalized():
        nc.finalize()   # Bacc.finalize -> compile(): register DCE/alloc etc.
    return nc


# ----------------------------------------------------------------------------
# Public entry point
# ----------------------------------------------------------------------------
_CACHE = {}


def kernel(**inputs):
    from concourse.bass_utils import run_bass_kernel_spmd

    import ml_dtypes
    inp = {k: np.asarray(v, dtype=np.float32) for k, v in inputs.items()}
    plan, (wba, wbb), bblob, _ = build_plan(inp)
    wba = tf32_round(wba)
    wbb = wbb.astype(ml_dtypes.bfloat16)
    nc = emit_bass(plan, (wba.shape[1], wbb.shape[1]), bblob.shape[1])

    x = inp["x"].reshape(B_TOTAL, XF)
    xT = tf32_round(np.ascontiguousarray(x.T))           # [121, B_TOTAL]
    in_maps = []
    for c in range(N_CORES):
        xc = np.ascontiguousarray(xT[:, c * B_CORE:(c + 1) * B_CORE])
        in_maps.append({"x": xc, "wb": wba, "wb2": wbb, "bb": bblob})
    res = run_bass_kernel_spmd(nc, in_maps, list(range(N_CORES)))
    global LAST_RESULTS, LAST_EXEC_NS
    LAST_RESULTS = res
    LAST_EXEC_NS = res.exec_time_ns
    outs = [res.results[c]["y"] for c in range(N_CORES)]  # [2, B_CORE] each
    y = np.concatenate(outs, axis=1).T                    # [B_TOTAL, 2]
    return np.ascontiguousarray(y, dtype=np.float32)


# ----------------------------------------------------------------------------
# Benchmarking helpers (repeated PJRT execution with device-resident inputs)
# ----------------------------------------------------------------------------
def _make_sharded_fn(nc):
    import jax
    import numpy as _np
    from jax.sharding import Mesh, PartitionSpec
    from jax.experimental.shard_map import shard_map
    import concourse.bass2jax as B2J
    import concourse.mybir as mybir

    B2J.install_neuronx_cc_hook()
    partition_name = nc.partition_id_tensor.name if nc.partition_id_tensor else None
    in_names, out_names, out_avals, zero_outs = [], [], [], []
    for alloc in nc.m.functions[0].allocations:
        if not isinstance(alloc, mybir.MemoryLocationSet):
            continue
        name = alloc.memorylocations[0].name
        if alloc.kind == "ExternalInput":
            if name != partition_name:
                in_names.append(name)
        elif alloc.kind == "ExternalOutput":
            out_names.append(name)
            shape = tuple(alloc.tensor_shape)
            dtype = mybir.dt.np(alloc.dtype)
            out_avals.append(jax.core.ShapedArray(shape, dtype))
            zero_outs.append(_np.zeros(shape, dtype))
    n_params = len(in_names)
    n_outs = len(out_avals)
    all_in = list(in_names) + list(out_names)
    if partition_name is not None:
        all_in.append(partition_name)

    def _body(*args):
        operands = list(args)
        if partition_name is not None:
            operands.append(B2J.partition_id_tensor())
        outs = B2J._bass_exec_p.bind(
            *operands, out_avals=tuple(out_avals), in_names=tuple(all_in),
            out_names=tuple(out_names), lowering_input_output_aliases=(),
            sim_require_finite=True, sim_require_nnan=True, nc=nc)
        return tuple(outs)

    devices = jax.devices()[:N_CORES]
    mesh = Mesh(np.asarray(devices), ("core",))
    in_specs = (PartitionSpec("core"),) * (n_params + n_outs)
    out_specs = (PartitionSpec("core"),) * n_outs
    donate = tuple(range(n_params, n_params + n_outs))
    fn = jax.jit(shard_map(_body, mesh=mesh, in_specs=in_specs,
                           out_specs=out_specs, check_rep=False),
                 donate_argnums=donate, keep_unused=True)
    return fn, in_names, out_names, zero_outs, mesh


def bench(n_iters=20, **inputs):
    import time
    import jax
    from jax.sharding import NamedSharding, PartitionSpec

    inp = {k: np.asarray(v, dtype=np.float32) for k, v in inputs.items()}
    plan, wblob, bblob, _ = build_plan(inp)
    wblob = tf32_round(wblob)
    nc = emit_bass(plan, wblob.shape[1], bblob.shape[1])

    x = inp["x"].reshape(B_TOTAL, XF)
    xT = tf32_round(np.ascontiguousarray(x.T))
    per_core = {"x": [np.ascontiguousarray(xT[:, c * B_CORE:(c + 1) * B_CORE])
                      for c in range(N_CORES)],
                "wb": [wblob] * N_CORES, "bb": [bblob] * N_CORES}

    times = []
    out = None
    dev_ins = None
    for it in range(n_iters):
        fn, in_names, out_names, zero_outs, mesh = _make_sharded_fn(nc)
        sh = NamedSharding(mesh, PartitionSpec("core"))
        if dev_ins is None:
            dev_ins = [jax.device_put(
                np.concatenate(per_core[name], axis=0), sh)
                for name in in_names]
        zo = [jax.device_put(np.concatenate([z] * N_CORES, axis=0), sh)
              for z in zero_outs]
        jax.block_until_ready(zo)
        out = fn(*dev_ins, *zo)      # includes jit+load on each fresh fn
        jax.block_until_ready(out)
        t0 = time.perf_counter()
        out2 = fn(*dev_ins, *[jax.device_put(
            np.concatenate([z] * N_CORES, axis=0), sh) for z in zero_outs])
        jax.block_until_ready(out2)
        times.append(time.perf_counter() - t0)
        out = out2
    ys = np.asarray(out[0])
    y = np.concatenate(np.split(ys, N_CORES, axis=0), axis=1).T
    return np.ascontiguousarray(y, np.float32), times



# revision 17
# speedup vs baseline: 1.4636x; 1.4636x over previous
"""BraggNN Trainium2 kernel (8-core data-parallel, Bass/Tile), fp8 DoubleRow.

Strategy (v2):
  - Feature-major layout: features on SBUF partitions, batch on the free dim.
  - Every conv matmul runs in fp8e4m3 DoubleRow mode: one TensorE
    instruction contracts TWO 128-row K-tiles at 0.5 cycles/row.  Moving
    operands that must pair live in shared "arena" tiles [128, NSLOT, BT]
    so a single strided 3-D access pattern can span both slots.
  - Biases are folded into the matmul weights via a constant-1.0 row of the
    (padded) x tile, and a constant-ones H-arena slot for conv2 (which also
    evens conv2's K-tile count to 10 = 5 clean pairs).
  - Scales keep every fp8 tensor in the normal range: W_G,W_1 x64 (so ag,
    h carry x64), ONES/EXP x1/8 (rcp ~0.9).  Leaky-ReLU commutes with the
    positive x64 so h/c2 evacuate via one scalar_tensor_tensor
    max(0.01*p, p) with no rescale; the x64 is divided out at the c3/dense
    ACT evacs (which is also where per-feature biases reappear as ACT bias
    APs).
  - softmax over W: exp on ACT into the fp8 es arena; row-sums via fp8
    DoubleRow ones-matmuls into 3 psum groups (96 rows = 3 spatial rows x
    32ch); reciprocal on DVE straight to fp8; the expansion back to s-space
    is an SBUF->SBUF DMA partition-broadcast (DMA engines are otherwise
    idle), which keeps ag SBUF-only so GPSIMD can compute it.
  - dense head stays bf16 (fp8 there is the one thing that hurts accuracy).
"""

import os
import sys

for _p in ("/opt/trn_rl_repo", "/root/.axon_site/_ro/trn_rl_repo"):
    if os.path.isdir(_p) and _p not in sys.path:
        sys.path.insert(0, _p)

import numpy as np
import ml_dtypes

F8NP = ml_dtypes.float8_e4m3      # TRN fp8_e4m3 (max 240)
BF16NP = ml_dtypes.bfloat16

# ----------------------------------------------------------------------------
# Geometry (hardcoded for BraggNN: x [B,1,11,11], B=16384)
# ----------------------------------------------------------------------------
B_TOTAL = 16384
N_CORES = 8
B_CORE = B_TOTAL // N_CORES          # 2048
BT = int(os.environ.get("KBT", "512"))   # batch tile (free dim per op)
NBT = B_CORE // BT

# grid1 / h-space: conv1 output 9x9, padded cols 9->10 => 90 positions, 64 ch
G1_R, G1_C, G1_CP = 9, 9, 10
NPOS1 = G1_R * G1_CP                  # 90
HF = NPOS1 * 64                       # 5760 features
HT = HF // 128                        # 45 h-tiles (2 positions each)

# s-space: NLB inter space, 32 ch over grid1
SF = NPOS1 * 32                       # 2880
ST = SF // 128                        # 22.5 -> use ceil
ST = (SF + 127) // 128                # 23 s-tiles (4 positions each)

# sums space: 3 row-groups of 96 (3 spatial rows x 32 ch)
NG = 3

# grid2 / conv2 out: 7x7 valid, padded cols 7->8 => 56 positions, 32 ch
G2_R, G2_C, G2_CP = 7, 7, 8
NPOS2 = G2_R * G2_CP                  # 56
C2F = NPOS2 * 32                      # 1792
C2T = C2F // 128                      # 14 c2-tiles (4 positions each)

# grid3 / conv3 out: 5x5 valid, padded cols 5->6 => 30 positions, 8 ch
G3_R, G3_C, G3_CP = 5, 5, 6
NPOS3 = G3_R * G3_CP                  # 30
C3F = NPOS3 * 8                       # 240
C3T = 2                               # c3 tiles [128, 112->pad 128]

XF = 121                              # input features 11*11
XROW_BIAS = 121                       # constant-1.0 row in the padded x tile

SC_G = 64.0                           # scale on W_G / W_1 (ag, h carry x64)
SC_S = 1.0 / 8.0                      # scale on ONES (rcp = 8/sums ~ 0.9)

# Arena slot maps
XAG_X0, XAG_X1 = 0, 1                 # two copies of x (tpg hi/lo pairs)
XAG_AG0 = 2                           # ag_u at slot 2+u
XAG_NSLOT = 2 + ST                    # 25
H_X = HT                              # copy of x in H arena (conv2 lin path)
H_NSLOT = HT + 1                      # 46


def _p1(i, j):
    return i * G1_CP + j


def _p2(i, j):
    return i * G2_CP + j


def _p3(i, j):
    return i * G3_CP + j


def q8(a):
    return np.asarray(a, dtype=np.float32).astype(F8NP)


def q8f(a):
    return q8(a).astype(np.float32)


# ----------------------------------------------------------------------------
# Host-side construction of all full (dense) layer matrices + bias vectors
# ----------------------------------------------------------------------------
def build_full_mats(inp):
    w1, b1 = inp["w1"], inp["b1"]          # [64,1,3,3], [64]
    wt, bt = inp["wt"][:, :, 0, 0], inp["bt"]
    wp, bp = inp["wp"][:, :, 0, 0], inp["bp"]
    wg, bg = inp["wg"][:, :, 0, 0], inp["bg"]
    wo, bo = inp["wo"][:, :, 0, 0], inp["bo"]
    w2, b2 = inp["w2"], inp["b2"]          # [32,64,3,3]
    w3, b3 = inp["w3"], inp["b3"]          # [8,32,3,3]

    M = {}
    # conv1: x [121] -> h [5760]
    W1 = np.zeros((XF, HF), np.float32)
    bh = np.zeros(HF, np.float32)
    for i in range(G1_R):
        for j in range(G1_C):
            p = _p1(i, j) * 64
            bh[p:p + 64] = b1 + bo
            for ki in range(3):
                for kj in range(3):
                    W1[(i + ki) * 11 + (j + kj), p:p + 64] = w1[:, 0, ki, kj]
    M["W1"], M["bh"] = W1, bh

    # composed theta/phi/g: x [121] -> s [2880]; eff 3x3 conv with 32 out ch
    for name, wmat, bvec in (("T", wt, bt), ("P", wp, bp), ("G", wg, bg)):
        wcomp = np.einsum("oc,ckl->okl", wmat, w1[:, 0])   # [32,3,3]
        beff = bvec + wmat @ b1                             # [32]
        Wf = np.zeros((XF, SF), np.float32)
        bf = np.zeros(SF, np.float32)
        for i in range(G1_R):
            for j in range(G1_C):
                p = _p1(i, j) * 32
                bf[p:p + 32] = beff
                for ki in range(3):
                    for kj in range(3):
                        Wf[(i + ki) * 11 + (j + kj), p:p + 32] = wcomp[:, ki, kj]
        M["W" + name] = Wf
        M["b" + name] = bf

    # ones for row sums: s [2880] -> sums in 3 row-groups of 96 (rowmod3 x 32)
    ONES = np.zeros((SF, NG, 96), np.float32)
    for i in range(G1_R):
        for j in range(G1_C):
            for c in range(32):
                ONES[_p1(i, j) * 32 + c, i // 3, (i % 3) * 32 + c] = 1.0
    M["ONES"] = ONES

    # wo: ag [2880] -> h [5760] (1x1)
    WO = np.zeros((SF, HF), np.float32)
    for i in range(G1_R):
        for j in range(G1_C):
            p = _p1(i, j)
            WO[p * 32:p * 32 + 32, p * 64:p * 64 + 64] = wo.T
    M["WO"] = WO

    # conv2: h [5760] -> c2 [1792]
    W2 = np.zeros((HF, C2F), np.float32)
    b2f = np.zeros(C2F, np.float32)
    for i in range(G2_R):
        for j in range(G2_C):
            p = _p2(i, j) * 32
            b2f[p:p + 32] = b2
            for ki in range(3):
                for kj in range(3):
                    q = _p1(i + ki, j + kj) * 64
                    W2[q:q + 64, p:p + 32] = w2[:, :, ki, kj].T
    M["W2"], M["b2"] = W2, b2f

    # conv3: c2 [1792] -> c3 [240]
    W3 = np.zeros((C2F, C3F), np.float32)
    b3f = np.zeros(C3F, np.float32)
    for i in range(G3_R):
        for j in range(G3_C):
            p = _p3(i, j) * 8
            b3f[p:p + 8] = b3
            for ki in range(3):
                for kj in range(3):
                    q = _p2(i + ki, j + kj) * 32
                    W3[q:q + 32, p:p + 8] = w3[:, :, ki, kj].T
    M["W3"], M["b3"] = W3, b3f

    # dense head; dw1 permuted from torch (c,i,j) flatten to our padded layout
    D1 = np.zeros((C3F, 64), np.float32)
    for c in range(8):
        for i in range(G3_R):
            for j in range(G3_C):
                D1[_p3(i, j) * 8 + c, :] = inp["dw1"][:, c * 25 + i * 5 + j]
    M["D1"] = D1
    M["D2"] = inp["dw2"].T.copy()
    M["D3"] = inp["dw3"].T.copy()
    M["D4"] = inp["dw4"].T.copy()          # [16, 8]
    M["D5"] = inp["dw5"].T.copy()          # [8, 2]
    for k in range(1, 6):
        M["bd%d" % k] = inp["db%d" % k].astype(np.float32)
    return M


# ----------------------------------------------------------------------------
# fp8 pair bank: each entry is a [128, 2, 128] DoubleRow stationary block
# ----------------------------------------------------------------------------
class PairBank:
    def __init__(self):
        self.pairs = []          # list of np [128, 256] fp8
        self.index = {}

    def add(self, blkA, blkB):
        """blkA/blkB: [K<=128, M<=128] float32 (pre-scaled). Returns pid."""
        def pad(b):
            p = np.zeros((128, 128), np.float32)
            p[:b.shape[0], :b.shape[1]] = b
            return q8(p)
        a, b = pad(blkA), pad(blkB)
        flat = np.concatenate([a, b], axis=1)   # [128, 256] fp8
        key = flat.tobytes()
        hit = self.index.get(key)
        if hit is not None:
            return hit
        pid = len(self.pairs)
        self.pairs.append(flat)
        self.index[key] = pid
        return pid

    def blob(self):
        if not self.pairs:
            return np.zeros((128, 0), F8NP)
        return np.concatenate(self.pairs, axis=1)   # [128, NP*256] fp8


class BfBank:
    """bf16 single blocks [128, M] for the dense head."""

    def __init__(self):
        self.cols = []
        self.total = 0
        self.index = {}

    def add(self, blk):
        K, Mm = blk.shape
        b = np.zeros((128, Mm), np.float32)
        b[:K] = blk
        b = b.astype(BF16NP)
        key = (Mm, b.tobytes())
        hit = self.index.get(key)
        if hit is not None:
            return hit
        ent = (self.total, K, Mm)
        self.cols.append(b)
        self.total += Mm
        self.index[key] = ent
        return ent

    def blob(self):
        if not self.cols:
            return np.zeros((128, 0), BF16NP)
        return np.concatenate(self.cols, axis=1)


class BiasBank:
    def __init__(self):
        self.cols = []
        self.index = {}

    def add(self, vec):
        P = vec.shape[0]
        key = (P, vec.tobytes())
        hit = self.index.get(key)
        if hit is not None:
            return hit
        pad = np.zeros(128, np.float32)
        pad[:P] = vec
        ent = (len(self.cols), P)
        self.cols.append(pad)
        self.index[key] = ent
        return ent

    def blob(self):
        return (np.stack(self.cols, axis=1) if self.cols
                else np.zeros((128, 1), np.float32))


def hilo(blk):
    """Split fp32 block into fp8 hi + fp8 lo (returned as fp32 for PairBank)."""
    hi = q8f(blk)
    lo = blk - hi
    return hi, lo


# ----------------------------------------------------------------------------
# Plan construction
# ----------------------------------------------------------------------------
def build_plan(inp):
    M = build_full_mats(inp)
    pb = PairBank()
    bb = BfBank()
    bias = BiasBank()
    P = {"M": M}

    # --- tpg: per s-tile u, 3 DoubleRows (W hi/lo on x,x) -------------------
    # extended weights [128, SF]: rows 0..120 x, row 121 bias
    for name, scale in (("T", 1.0), ("P", 1.0), ("G", SC_G)):
        Wx = np.zeros((128, ST * 128), np.float32)
        Wx[:XF, :SF] = M["W" + name] * scale
        Wx[XROW_BIAS, :SF] = M["b" + name] * scale
        ents = []
        for u in range(ST):
            hi, lo = hilo(Wx[:, u * 128:(u + 1) * 128])
            ents.append(pb.add(hi, lo))
        P["tpg" + name] = ents

    # --- ones: 3 groups, 4 pairs each over 8 consecutive es k-tiles ---------
    # ONES8[sf, g, 96] -> blocks per (u, g): [128, 128] (96 cols used)
    ONES8 = M["ONES"] * SC_S
    ones_plan = []          # per g: list of (pid, slotA, slotB)
    for g in range(NG):
        Og = np.zeros((ST * 128, 128), np.float32)
        Og[:SF, :96] = ONES8[:, g, :]
        us = [u for u in range(ST)
              if np.any(Og[u * 128:(u + 1) * 128])]
        assert us == list(range(min(us), min(us) + len(us))), (g, us)
        if len(us) % 2:
            us.append(us[-1] + 1 if us[-1] + 1 < ST else us[0] - 1)
            us.sort()
        prs = []
        for a in range(0, len(us), 2):
            ua, ub = us[a], us[a + 1]
            pid = pb.add(Og[ua * 128:(ua + 1) * 128],
                         Og[ub * 128:(ub + 1) * 128])
            prs.append((pid, ua, ub))
        ones_plan.append(prs)
    P["ones"] = ones_plan

    # --- conv1 + wo fused: per h-tile m, one DoubleRow ----------------------
    # slot A: x (with bias row = SC_G*bh), slot B: ag_{m//2}
    W1x = np.zeros((128, HF), np.float32)
    W1x[:XF] = M["W1"] * SC_G
    W1x[XROW_BIAS] = M["bh"] * SC_G
    ents = []
    for m in range(HT):
        u = m // 2
        wo_blk = M["WO"][u * 128:(u + 1) * 128, m * 128:(m + 1) * 128]
        pid = pb.add(W1x[:, m * 128:(m + 1) * 128], wo_blk)
        ents.append((pid, u))
    P["c1wo"] = ents

    # --- conv2 (relu-split): lrelu(h) = 0.99*relu(h) + 0.01*h; the linear
    # term composes through conv1 into a single x K-tile (x's constant-1 row
    # also carries b2 and the composed bh leak); the 0.01*W2*WO*ag cross term
    # (~1e-4 relative) is dropped.  10 K-tiles -> 5 clean DoubleRows.
    # XC = (64*W1 incl bias row) @ (0.01*W2), row121 += 64*b2
    XC = W1x @ (0.01 * M["W2"])                     # [128, C2F]
    XC[XROW_BIAS] += SC_G * M["b2"]
    conv2_plan = []
    for ot in range(C2T):
        r0, t0 = ot // 2, 2 * (ot % 2)
        ks = [5 * (r0 + r) + t0 + dt for r in range(3) for dt in range(3)]
        pairs = [(ks[0], ks[1]), (ks[3], ks[4]), (ks[6], ks[7]),
                 (ks[2], ks[5]), (ks[8], H_X)]
        prs = []
        for (ka, kb) in pairs:
            def blk(k):
                if k == H_X:
                    return XC[:, ot * 128:(ot + 1) * 128]
                return 0.99 * M["W2"][k * 128:(k + 1) * 128,
                                      ot * 128:(ot + 1) * 128]
            assert ka < kb, (ka, kb)
            pid = pb.add(blk(ka), blk(kb))
            prs.append((pid, ka, kb))
        conv2_plan.append(prs)
    P["conv2"] = conv2_plan

    # --- conv3: per c3-tile, 5 DoubleRows over 10 adjacent c2-tiles ---------
    # h' carries x64 -> psum = 64*c3pre; bias at ACT evac.
    W3p = np.zeros((C2T * 128, C3T * 128), np.float32)
    W3p[:C2F, :C3F] = M["W3"]
    conv3_plan = []
    for ot in range(C3T):
        ks = [k for k in range(C2T)
              if np.any(W3p[k * 128:(k + 1) * 128,
                            ot * 128:(ot + 1) * 128])]
        assert ks == list(range(min(ks), min(ks) + len(ks))), ks
        if len(ks) % 2:
            ks.append(ks[-1] + 1 if ks[-1] + 1 < C2T else ks[0] - 1)
            ks.sort()
        prs = []
        for a in range(0, len(ks), 2):
            ka, kb = ks[a], ks[a + 1]
            pid = pb.add(W3p[ka * 128:(ka + 1) * 128,
                             ot * 128:(ot + 1) * 128],
                         W3p[kb * 128:(kb + 1) * 128,
                             ot * 128:(ot + 1) * 128])
            prs.append((pid, ka, kb))
        conv3_plan.append(prs)
    P["conv3"] = conv3_plan
    b3p = np.zeros(C3T * 128, np.float32)
    b3p[:C3F] = M["b3"]
    P["bias3"] = [bias.add(b3p[lo:lo + 128]) for lo in range(0, C3T * 128, 128)]

    # --- dense head (bf16) --------------------------------------------------
    P["d1"] = [bb.add(M["D1"][k * 128:min((k + 1) * 128, C3F), :])
               for k in range(C3T)]
    P["d2"] = [bb.add(M["D2"])]
    P["d3"] = [bb.add(M["D3"])]
    P["d4"] = [bb.add(M["D4"])]
    P["d5"] = [bb.add(M["D5"])]
    for k in range(1, 6):
        P["biasd%d" % k] = bias.add(M["bd%d" % k])

    # --- rcp broadcast schedule: per u, list of (dst_lo, dst_n32, g, rowmod)
    bcast = []
    for u in range(ST):
        segs = []
        p0 = 4 * u
        k = 0
        while k < 4:
            pos = p0 + k
            i = pos // G1_CP
            n = 1
            while k + n < 4 and (p0 + k + n) // G1_CP == i:
                n += 1
            if i < G1_R:
                segs.append((k * 32, n, i // 3, i % 3))
            k += n
        bcast.append(segs)
    P["bcast"] = bcast

    return P, pb.blob(), bb.blob(), bias.blob()


# ----------------------------------------------------------------------------
# Numpy forward replicating the exact plan semantics (layout validator)
# ----------------------------------------------------------------------------
def np_forward(P, w8, wbf, bblob, xq):
    """xq: [128, N] fp8-quantized padded input (row 121 = 1). Returns [2, N]."""
    f32 = np.float32
    w8f = w8.astype(f32)
    wbff = wbf.astype(f32)
    N = xq.shape[1]
    xf = xq.astype(f32)

    def dr(pid, a, b):
        W = w8f[:, pid * 256:(pid + 1) * 256]
        return W[:, :128].T @ a + W[:, 128:].T @ b

    # tpg
    tp = {}
    for nm in ("T", "P", "G"):
        outs = []
        for u in range(ST):
            outs.append(dr(P["tpg" + nm][u], xf, xf))
        tp[nm] = np.concatenate(outs, axis=0)      # [ST*128, N]
    s = tp["T"] * tp["P"]
    es = np.zeros((ST * 128, N), f32)
    es[:] = q8f(np.exp(s))
    # ones
    sums = np.zeros((NG, 128, N), f32)
    for g in range(NG):
        for (pid, ua, ub) in P["ones"][g]:
            sums[g] += dr(pid, es[ua * 128:(ua + 1) * 128],
                          es[ub * 128:(ub + 1) * 128])
    rcp = q8f(1.0 / sums[:, :96, :])               # [NG, 96, N]
    # broadcast to ep, ag
    ag = np.zeros((ST * 128, N), f32)
    for u in range(ST):
        ep = np.zeros((128, N), f32)
        for (dlo, n32, g, rmod) in P["bcast"][u]:
            for r in range(n32):
                ep[dlo + r * 32:dlo + (r + 1) * 32] = \
                    rcp[g, rmod * 32:(rmod + 1) * 32]
        a1 = es[u * 128:(u + 1) * 128] * tp["G"][u * 128:(u + 1) * 128]
        ag[u * 128:(u + 1) * 128] = q8f(q8f(a1.astype(BF16NP).astype(f32))
                                        * ep)
    # conv1 + wo -> h (relu evac; linear lrelu leak flows via conv2's XC)
    hq = np.zeros((HT * 128, N), f32)
    for m in range(HT):
        pid, u = P["c1wo"][m]
        ps = dr(pid, xf, ag[u * 128:(u + 1) * 128])
        hq[m * 128:(m + 1) * 128] = q8f(np.maximum(ps, 0.0))
    # conv2
    c2q = np.zeros((C2T * 128, N), f32)
    for ot in range(C2T):
        ps = np.zeros((128, N), f32)
        for (pid, ka, kb) in P["conv2"][ot]:
            a = xf if ka == H_X else hq[ka * 128:(ka + 1) * 128]
            b = xf if kb == H_X else hq[kb * 128:(kb + 1) * 128]
            ps += dr(pid, a, b)
        c2q[ot * 128:(ot + 1) * 128] = q8f(np.maximum(0.01 * ps, ps))
    # conv3 (psum = 64*c3pre), ACT evac scale 1/64 + bias -> bf16
    lrelu = lambda v: np.where(v >= 0, v, 0.01 * v)
    c3 = np.zeros((C3T * 128, N), f32)
    for ot in range(C3T):
        ps = np.zeros((128, N), f32)
        for (pid, ka, kb) in P["conv3"][ot]:
            ps += dr(pid, c2q[ka * 128:(ka + 1) * 128],
                     c2q[kb * 128:(kb + 1) * 128])
        col, Pn = P["bias3"][ot]
        b = bblob[:, col]
        c3[ot * 128:(ot + 1) * 128] = lrelu(ps / SC_G + b[:, None]) \
            .astype(BF16NP).astype(f32)

    # dense head bf16
    def bmm(name, z):
        acc = 0.0
        for ki, (off, K, Mm) in enumerate(P[name]):
            W = wbff[:, off:off + Mm]
            acc = acc + W.T @ z[ki * 128:(ki + 1) * 128][:128][:W.shape[0]]
        return acc

    def dn(name, bname, z, act=True):
        acc = 0.0
        for ki, (off, K, Mm) in enumerate(P[name]):
            W = wbff[:, off:off + Mm]
            acc = acc + W.T @ z[ki * 128:ki * 128 + 128]
        col, Pn = P[bname]
        r = acc + bblob[:Mm, col][:, None]
        if act:
            r = lrelu(r).astype(BF16NP).astype(f32)
        return r

    z = dn("d1", "biasd1", c3)
    z = dn("d2", "biasd2", np.pad(z, ((0, 128 - z.shape[0]), (0, 0))))
    z = dn("d3", "biasd3", np.pad(z, ((0, 128 - z.shape[0]), (0, 0))))
    z = dn("d4", "biasd4", np.pad(z, ((0, 128 - z.shape[0]), (0, 0))))
    z = dn("d5", "biasd5", np.pad(z, ((0, 128 - z.shape[0]), (0, 0))),
           act=False)
    return z[:2]


# ----------------------------------------------------------------------------
# Bass kernel emission
# ----------------------------------------------------------------------------
DBG_LOOP = 0           # device-side repeat count for benchmarking
H_BUFS = 2             # H arena double-buffer depth

# engine assignment for the flexible evacs: first HE_ACT h-tiles on ACT,
# rest on DVE; c2 split likewise
import json as _json
TUNE = {"h_act": 24, "ag_eng": "gps", "a1_eng": "dve", "phc_eng": "act",
        "xag": 2, "es": 2, "ep": 2, "h": 2, "c2": 2, "s": 4, "a1": 4,
        "phs": 4,
        "tpg_ps": 2, "mm": 1, "sums": 1, "rcp": 2, "c3": 2, "z": 2}
if os.environ.get("KTUNE"):
    TUNE.update(_json.loads(os.environ["KTUNE"]))


def emit_bass(plan, n8cols, nbfcols, nbcols):
    import concourse.bacc as bacc
    import concourse.mybir as mybir
    from concourse.tile import TileContext

    F8 = mybir.dt.float8e4
    BF16 = mybir.dt.bfloat16
    F32 = mybir.dt.float32
    AF = mybir.ActivationFunctionType
    OP = mybir.AluOpType
    DR = mybir.MatmulPerfMode.DoubleRow
    P = plan

    nd = int(os.environ.get("DBG_ND", str(N_CORES)))
    nbt = int(os.environ.get("DBG_NBT", str(NBT)))
    nc = bacc.Bacc("TRN2", target_bir_lowering=True, debug=False,
                   num_devices=nd)
    NP8 = n8cols // 256
    x_d = nc.dram_tensor("x", [128, B_CORE], F8, kind="ExternalInput")
    w8_d = nc.dram_tensor("w8", [128, n8cols], F8, kind="ExternalInput")
    wbf_d = nc.dram_tensor("wbf", [128, nbfcols], BF16, kind="ExternalInput")
    b_d = nc.dram_tensor("bb", [128, nbcols], F32, kind="ExternalInput")
    y_d = nc.dram_tensor("y", [2, B_CORE], F32, kind="ExternalOutput")

    with TileContext(nc) as tc:
        with nc.allow_low_precision(reason="fp8 by design"), \
             tc.tile_pool(name="sb", bufs=1) as sb, \
             tc.tile_pool(name="ps", bufs=1, space="PSUM") as psp:

            # ---- weights/biases resident in SBUF ----
            w8sb = sb.tile([128, NP8 * 2, 128], F8, tag="w8", bufs=1)
            wbfsb = sb.tile([128, max(nbfcols, 1)], BF16, tag="wbf", bufs=1)
            bsb = sb.tile([128, nbcols], F32, tag="bsb", bufs=1)
            w8flat = w8sb.rearrange("p a b -> p (a b)")
            CH = 4096
            for lo in range(0, n8cols, CH):
                hi = min(lo + CH, n8cols)
                nc.sync.dma_start(out=w8flat[:, lo:hi], in_=w8_d[:, lo:hi])
            if nbfcols:
                nc.sync.dma_start(out=wbfsb[:, :nbfcols], in_=wbf_d[:])
            nc.sync.dma_start(out=bsb[:], in_=b_d[:])

            def wpair(pid):
                return w8sb[:, 2 * pid:2 * pid + 2, :]

            def wbf(ent):
                off, K, Mm = ent
                return wbfsb[0:K, off:off + Mm]

            def bap(ent):
                col, Pp = ent
                return bsb[0:Pp, col:col + 1]

            import contextlib as _ctx
            loop_cm = (tc.For_i(0, DBG_LOOP, 1,
                                hint_engines=(mybir.EngineType.PE,
                                              mybir.EngineType.Activation,
                                              mybir.EngineType.DVE))
                       if DBG_LOOP > 1 else _ctx.nullcontext())
            with loop_cm:
              for bt in range(nbt):
                bsl = slice(bt * BT, (bt + 1) * BT)
                xag = sb.tile([128, XAG_NSLOT, BT], F8, tag="xag",
                              bufs=TUNE["xag"], name="xag")
                esa = sb.tile([128, ST, BT], F8, tag="es", bufs=TUNE["es"],
                              name="esa")
                epa = sb.tile([128, ST, BT], F8, tag="ep", bufs=TUNE["ep"],
                              name="epa")
                ha = sb.tile([128, H_NSLOT, BT], F8, tag="h", bufs=TUNE["h"],
                             name="ha")
                c2a = sb.tile([128, C2T, BT], F8, tag="c2", bufs=TUNE["c2"],
                              name="c2a")
                rcp = sb.tile([96, NG, BT], F8, tag="rcp", bufs=TUNE["rcp"],
                              name="rcp")

                nc.sync.dma_start(out=xag[:, XAG_X0, :], in_=x_d[:, bsl])
                nc.sync.dma_start(out=xag[:, XAG_X1, :], in_=x_d[:, bsl])
                nc.sync.dma_start(out=ha[:, H_X, :], in_=x_d[:, bsl])

                def pairsl(arena, a, b):
                    assert a < b, (a, b)
                    return arena[:, a:b + 1:b - a, :]

                xx = xag[:, 0:2, :]

                gps = [None] * ST
                h_done = [False] * HT
                c2_done = [False] * C2T
                sums_ps = [None] * NG
                rcp_done = [False] * NG
                ag_done = [False] * ST
                n_h_act = [0]
                n_c2_act = [0]

                # --- phase T: tpg + exp ---
                def emit_tpg(u):
                    tps = psp.tile([128, BT], F32, tag="tpgT",
                                   bufs=TUNE["tpg_ps"], name="tps")
                    nc.tensor.matmul(tps[:], wpair(P["tpgT"][u]), xx,
                                     start=True, stop=True, perf_mode=DR)
                    pps = psp.tile([128, BT], F32, tag="tpgP",
                                   bufs=TUNE["tpg_ps"], name="pps")
                    nc.tensor.matmul(pps[:], wpair(P["tpgP"][u]), xx,
                                     start=True, stop=True, perf_mode=DR)
                    gp = psp.tile([128, BT], F32, tag="tpgG",
                                  bufs=TUNE["tpg_ps"], name="gps")
                    nc.tensor.matmul(gp[:], wpair(P["tpgG"][u]), xx,
                                     start=True, stop=True, perf_mode=DR)
                    gps[u] = gp
                    # phi: psum -> SBUF copy (DVE reads only one PSUM operand)
                    phs = sb.tile([128, BT], BF16, tag="phs",
                                  bufs=TUNE["phs"], name="phs")
                    if TUNE["phc_eng"] == "act":
                        nc.scalar.activation(phs[:], pps[:], AF.Copy)
                    else:
                        nc.vector.tensor_copy(phs[:], pps[:])
                    s_sb = sb.tile([128, BT], BF16, tag="s", bufs=TUNE["s"],
                                   name="s_sb")
                    nc.vector.tensor_tensor(out=s_sb[:], in0=tps[:],
                                            in1=phs[:], op=OP.mult)
                    nc.scalar.activation(esa[:, u, :], s_sb[:], AF.Exp)
                    a1 = sb.tile([128, BT], BF16, tag="a1", bufs=TUNE["a1"],
                                 name="a1")
                    a1_eng = (nc.gpsimd if TUNE["a1_eng"] == "gps"
                              else nc.vector)
                    a1_eng.tensor_tensor(out=a1[:], in0=esa[:, u, :],
                                         in1=gps[u][:], op=OP.mult)
                    gps[u] = a1     # repurpose: holds a1 now

                # --- sums + rcp per group ---
                def emit_sums(g):
                    sp = psp.tile([128, BT], F32, tag="sums",
                                  bufs=TUNE["sums"], name="sums")
                    prs = P["ones"][g]
                    for i, (pid, ua, ub) in enumerate(prs):
                        nc.tensor.matmul(sp[:], wpair(pid),
                                         pairsl(esa, ua, ub),
                                         start=(i == 0),
                                         stop=(i == len(prs) - 1),
                                         perf_mode=DR)
                    nc.vector.reciprocal(rcp[:, g, :], sp[0:96, :])
                    rcp_done[g] = True

                # --- ep broadcast + ag per s-tile ---
                def emit_ag(u):
                    for (dlo, n32, g, rmod) in P["bcast"][u]:
                        src = rcp[rmod * 32:(rmod + 1) * 32, g, :]
                        for r in range(n32):
                            nc.sync.dma_start(
                                out=epa[dlo + r * 32:dlo + (r + 1) * 32,
                                        u, :],
                                in_=src)
                    ag_eng = (nc.gpsimd if TUNE["ag_eng"] == "gps"
                              else nc.vector)
                    ag_eng.tensor_tensor(out=xag[:, XAG_AG0 + u, :],
                                         in0=gps[u][:], in1=epa[:, u, :],
                                         op=OP.mult)
                    ag_done[u] = True

                # --- conv1+wo + h evac (relu; split across ACT/DVE) ---
                def emit_h(m):
                    pid, u = P["c1wo"][m]
                    hp = psp.tile([128, BT], F32, tag="mm", bufs=TUNE["mm"],
                                  name="hps")
                    nc.tensor.matmul(hp[:], wpair(pid),
                                     pairsl(xag, XAG_X0, XAG_AG0 + u),
                                     start=True, stop=True, perf_mode=DR)
                    if n_h_act[0] < TUNE["h_act"]:
                        n_h_act[0] += 1
                        nc.scalar.activation(ha[:, m, :], hp[:], AF.Relu)
                    else:
                        nc.vector.tensor_scalar_max(ha[:, m, :], hp[:], 0.0)
                    h_done[m] = True

                # --- conv2 per c2-tile when inputs ready ---
                def emit_c2_ready():
                    for ot in range(C2T):
                        if c2_done[ot]:
                            continue
                        ks = set()
                        for (pid, ka, kb) in P["conv2"][ot]:
                            ks.update((ka, kb))
                        ks.discard(H_X)
                        if not all(h_done[k] for k in ks):
                            continue
                        cp = psp.tile([128, BT], F32, tag="mm",
                                      bufs=TUNE["mm"], name="c2ps")
                        prs = P["conv2"][ot]
                        for i, (pid, ka, kb) in enumerate(prs):
                            nc.tensor.matmul(cp[:], wpair(pid),
                                             pairsl(ha, ka, kb),
                                             start=(i == 0),
                                             stop=(i == len(prs) - 1),
                                             perf_mode=DR)
                        nc.scalar.activation(c2a[:, ot, :], cp[:],
                                             AF.Lrelu, alpha=0.01)
                        c2_done[ot] = True

                # ---------------- schedule ----------------
                # group boundaries: group g's ones need es for u in ug[g]
                ug_hi = [max(ub for (_, _, ub) in P["ones"][g])
                         for g in range(NG)]
                # u's ag needs rcp groups from bcast
                u_gmax = [max(g for (_, _, g, _) in P["bcast"][u])
                          for u in range(ST)]

                next_ag = [0]

                def drain_ag():
                    while (next_ag[0] < ST
                           and all(rcp_done[g] for g in
                                   range(u_gmax[next_ag[0]] + 1))):
                        u = next_ag[0]
                        emit_ag(u)
                        emit_h(2 * u)
                        if 2 * u + 1 < HT:
                            emit_h(2 * u + 1)
                        next_ag[0] += 1

                for u in range(ST):
                    emit_tpg(u)
                    for g in range(NG):
                        if not rcp_done[g] and ug_hi[g] == u:
                            emit_sums(g)
                            drain_ag()
                            emit_c2_ready()
                drain_ag()
                assert all(ag_done), ag_done
                emit_c2_ready()
                assert all(c2_done)

                # --- conv3 ---
                c3 = []
                for ot in range(C3T):
                    cp = psp.tile([128, BT], F32, tag="mm", bufs=TUNE["mm"],
                                  name="c3ps")
                    prs = P["conv3"][ot]
                    for i, (pid, ka, kb) in enumerate(prs):
                        nc.tensor.matmul(cp[:], wpair(pid),
                                         pairsl(c2a, ka, kb),
                                         start=(i == 0),
                                         stop=(i == len(prs) - 1),
                                         perf_mode=DR)
                    c3m = sb.tile([128, BT], BF16, tag="c3", bufs=TUNE["c3"],
                                  name="c3m")
                    nc.scalar.activation(c3m[:], cp[:], AF.Lrelu,
                                         bias=bap(P["bias3"][ot]),
                                         scale=1.0 / SC_G, alpha=0.01)
                    c3.append(c3m)

                # --- dense head (bf16) ---
                def dense(name, bname, rhs_tiles, Mo, func, dt=BF16):
                    dp = psp.tile([Mo, BT], F32, tag="mm", bufs=TUNE["mm"],
                                  name="dps")
                    ents = P[name]
                    for i, ent in enumerate(ents):
                        nc.tensor.matmul(dp[:], wbf(ent),
                                         rhs_tiles[i][0:ent[1], :],
                                         start=(i == 0),
                                         stop=(i == len(ents) - 1))
                    z = sb.tile([Mo, BT], dt, tag="z", bufs=TUNE["z"],
                                name="z" + name)
                    nc.scalar.activation(z[:], dp[:], func,
                                         bias=bap(P[bname]),
                                         alpha=0.01 if func == AF.Lrelu
                                         else 0.0)
                    return z

                z1 = dense("d1", "biasd1", c3, 64, AF.Lrelu)
                z2 = dense("d2", "biasd2", [z1], 32, AF.Lrelu)
                z3 = dense("d3", "biasd3", [z2], 16, AF.Lrelu)
                z4 = dense("d4", "biasd4", [z3], 8, AF.Lrelu)
                y_sb = dense("d5", "biasd5", [z4], 2, AF.Identity, dt=F32)
                nc.sync.dma_start(out=y_d[:, bsl], in_=y_sb[:])
    if not nc.is_finalized():
        nc.finalize()
    return nc


# ----------------------------------------------------------------------------
# Host-side input prep
# ----------------------------------------------------------------------------
def prep_x(x):
    """x: [B, 1, 11, 11] fp32 -> [128, B] fp8 padded, row 121 = 1.0."""
    B = x.shape[0]
    xT = np.zeros((128, B), np.float32)
    xT[:XF] = x.reshape(B, XF).T
    xT[XROW_BIAS] = 1.0
    return q8(np.ascontiguousarray(xT))


# ----------------------------------------------------------------------------
# Public entry point
# ----------------------------------------------------------------------------
def kernel(**inputs):
    from concourse.bass_utils import run_bass_kernel_spmd

    inp = {k: np.asarray(v, dtype=np.float32) for k, v in inputs.items()}
    plan, w8, wbf, bblob = build_plan(inp)
    nc = emit_bass(plan, w8.shape[1], wbf.shape[1], bblob.shape[1])

    xq = prep_x(inp["x"])                                # [128, B_TOTAL] fp8
    in_maps = []
    for c in range(N_CORES):
        xc = np.ascontiguousarray(xq[:, c * B_CORE:(c + 1) * B_CORE])
        in_maps.append({"x": xc, "w8": w8, "wbf": wbf, "bb": bblob})
    res = run_bass_kernel_spmd(nc, in_maps, list(range(N_CORES)))
    global LAST_RESULTS, LAST_EXEC_NS
    LAST_RESULTS = res
    LAST_EXEC_NS = res.exec_time_ns
    outs = [res.results[c]["y"] for c in range(N_CORES)]  # [2, B_CORE] each
    y = np.concatenate(outs, axis=1).T                    # [B_TOTAL, 2]
    return np.ascontiguousarray(y, dtype=np.float32)


# ----------------------------------------------------------------------------
# Benchmarking helpers (repeated PJRT execution with device-resident inputs)
# ----------------------------------------------------------------------------
def _make_sharded_fn(nc):
    import jax
    import numpy as _np
    from jax.sharding import Mesh, PartitionSpec
    from jax.experimental.shard_map import shard_map
    import concourse.bass2jax as B2J
    import concourse.mybir as mybir

    B2J.install_neuronx_cc_hook()
    partition_name = (nc.partition_id_tensor.name
                      if nc.partition_id_tensor else None)
    in_names, out_names, out_avals, zero_outs = [], [], [], []
    for alloc in nc.m.functions[0].allocations:
        if not isinstance(alloc, mybir.MemoryLocationSet):
            continue
        name = alloc.memorylocations[0].name
        if alloc.kind == "ExternalInput":
            if name != partition_name:
                in_names.append(name)
        elif alloc.kind == "ExternalOutput":
            out_names.append(name)
            shape = tuple(alloc.tensor_shape)
            dtype = mybir.dt.np(alloc.dtype)
            out_avals.append(jax.core.ShapedArray(shape, dtype))
            zero_outs.append(_np.zeros(shape, dtype))
    n_params = len(in_names)
    n_outs = len(out_avals)
    all_in = list(in_names) + list(out_names)
    if partition_name is not None:
        all_in.append(partition_name)

    def _body(*args):
        operands = list(args)
        if partition_name is not None:
            operands.append(B2J.partition_id_tensor())
        outs = B2J._bass_exec_p.bind(
            *operands, out_avals=tuple(out_avals), in_names=tuple(all_in),
            out_names=tuple(out_names), lowering_input_output_aliases=(),
            sim_require_finite=True, sim_require_nnan=True, nc=nc)
        return tuple(outs)

    devices = jax.devices()[:N_CORES]
    mesh = Mesh(np.asarray(devices), ("core",))
    in_specs = (PartitionSpec("core"),) * (n_params + n_outs)
    out_specs = (PartitionSpec("core"),) * n_outs
    donate = tuple(range(n_params, n_params + n_outs))
    fn = jax.jit(shard_map(_body, mesh=mesh, in_specs=in_specs,
                           out_specs=out_specs, check_rep=False),
                 donate_argnums=donate, keep_unused=True)
    return fn, in_names, out_names, zero_outs, mesh
